# revision 30
# baseline (speedup 1.0000x reference)
"""GRU cell (B=4096, H=2048) on 8 TRN2 NeuronCores — fp8/bf16 mixed.

Sharding: data-parallel over batch — each core computes 512 rows; weights
replicated, no collectives.

Per-core compute in transposed space (hidden on partitions, batch free).
All weights are pre-scaled on the host so max|w8| = 0.9995: the weights
are uniform(+-stdv), and placing the max just under a binade boundary
fills e4m3's finest relative grid (quantization MSE x0.73 vs a scale
that lands max|w8| mid-binade).  Every activation descales with 1/SW.

Precision schedule: r/z gates and the n-gate hh half are fully fp8-e4m3
DoubleRow (2 k-chunks per matmul; the hh error is attenuated by r inside
tanh(gi2 + r*gh2)).  The n-gate ih half is all-fp8 for hidden blocks
0-10 and 8 bf16 k-chunks + 8 fp8-DR chunks for blocks 11-15 (error
variance is linear in the bf16 chunk count, so concentrating the budget
in few blocks halves the FWL<->DoubleRow mode switches and keeps bf16
out of the HBM-bound startup window).  The hx used in the output blend
is fp32 (error margin at DMA-only cost).

Startup: the DMA engines ramp (~110 GB/s for the first ~5us under
8-core HBM contention, ~350 GB/s after), so block 0's operands land
~7us after the first transfer is issued.  A tiny memset tile feeds
FD=128 fp8-DR warm-up matmuls that bridge that whole window — idling
the PE >3.4us would reset the HAM clock ramp back to 1.2 GHz.  Blocks
0-1 are fused xt-side-first: all five xt8 sweeps run while hxt8 and
the hh-side slabs stream in behind them, so the hh sweeps start with
their data landed.  Weight slabs prefetch ~2 blocks ahead of use.

Last block: r/gh/gi early so the whole tanh chain runs during the z
sweeps; z is computed in 320/192 column parts into TWO PSUM tiles (PSUM
read deps are whole-tile) so the wide part's sigmoid/mul/add/DMA runs
under the short part's matmuls and only a 192-wide chain trails the
final matmul.  The first act transfer rides the scalar ring so its
queue first-byte latency overlaps the sync ring's.

Measured on HW: 190.7us, rel err 1.968e-2 (gate 2e-2); the numpy
emulation of the quantization error predicts the HW rel err to ~3-4
digits (emu 1.9684e-2).
"""

from contextlib import ExitStack

import ml_dtypes
import numpy as np

import concourse.bass as bass
import concourse.tile as tile
from concourse import bacc, mybir
from concourse.bass_utils import run_bass_kernel_spmd

H = 2048
B = 4096
N_CORES = 8
BL = B // N_CORES  # 512 batch rows per core
P = 128
NKB = H // P  # 16 contraction chunks
NNB = H // P  # 16 hidden (output) blocks
F32 = mybir.dt.float32
F8 = mybir.dt.float8e4
BF16 = mybir.dt.bfloat16
DR = mybir.MatmulPerfMode.DoubleRow
NBF = 8  # n-gate ih-half bf16 k-chunks (blocks BF0..15; rest fp8-DR)
NF8 = NKB - NBF
BF0 = 11  # first hidden block with the bf16 segment
NFP = NNB - BF0  # number of bf16-carrying blocks
SWMAX = 0.9995  # target max|w8| — just under the binade boundary

# w8 matrix order: 0 r_ih, 1 r_hh, 2 z_ih, 3 z_hh.  w8nf: full-K fp8
# n-ih for blocks 0..BF0-1.  w16/w8n: the n-ih split for blocks BF0+.
# w8nh: n-gate hh half, fully fp8.  b_hh2 is pre-scaled xSW so it can
# add to the SW-scaled PSUM before the tanh descale.


def _build_program(sw: float) -> bacc.Bacc:
    nc = bacc.Bacc(
        "TRN2", target_bir_lowering=False, debug=False, num_devices=N_CORES
    )

    xt8 = nc.dram_tensor("xt8", [P, NKB, BL], F8, kind="ExternalInput").ap()
    hxt8 = nc.dram_tensor("hxt8", [P, NKB, BL], F8, kind="ExternalInput").ap()
    xtb = nc.dram_tensor("xtb", [P, NBF, BL], BF16, kind="ExternalInput").ap()
    hxt32 = nc.dram_tensor("hxt32", [P, NKB, BL], F32, kind="ExternalInput").ap()
    w8 = nc.dram_tensor("w8", [4, NNB, P, NKB, P], F8, kind="ExternalInput").ap()
    w16 = nc.dram_tensor("w16", [NFP, P, NBF, P], BF16, kind="ExternalInput").ap()
    w8n = nc.dram_tensor("w8n", [NFP, P, NF8, P], F8, kind="ExternalInput").ap()
    w8nh = nc.dram_tensor("w8nh", [NNB, P, NKB, P], F8, kind="ExternalInput").ap()
    w8nf = nc.dram_tensor("w8nf", [BF0, P, NKB, P], F8, kind="ExternalInput").ap()
    b = nc.dram_tensor("b", [P, 5 * NNB], F32, kind="ExternalInput").ap()
    out = nc.dram_tensor("out", [H, BL], F32, kind="ExternalOutput").ap()

    with tile.TileContext(nc) as tc, ExitStack() as ctx:
        const = ctx.enter_context(tc.tile_pool(name="const", bufs=1))
        acts = ctx.enter_context(tc.tile_pool(name="acts", bufs=1))
        wp8 = ctx.enter_context(tc.tile_pool(name="wp8", bufs=24))
        wp8n = ctx.enter_context(tc.tile_pool(name="wp8n", bufs=4))
        wp16 = ctx.enter_context(tc.tile_pool(name="wp16", bufs=4))
        gates = ctx.enter_context(tc.tile_pool(name="gates", bufs=2))
        opool = ctx.enter_context(tc.tile_pool(name="opool", bufs=3))
        ps_r = ctx.enter_context(tc.tile_pool(name="ps_r", bufs=2, space="PSUM"))
        ps_z = ctx.enter_context(tc.tile_pool(name="ps_z", bufs=2, space="PSUM"))
        ps_gi = ctx.enter_context(tc.tile_pool(name="ps_gi", bufs=2, space="PSUM"))
        ps_gh = ctx.enter_context(tc.tile_pool(name="ps_gh", bufs=2, space="PSUM"))

        # PE warm-up bridge (see module docstring).
        warm = const.tile([P, 2, P], F8)
        nc.gpsimd.memset(warm[:], 0.0)
        p_warm = ps_gh.tile([P, BL], F32, tag="p_gh", name="p_warm")

        def warm_mms(n):
            for _ in range(n):
                nc.tensor.matmul(
                    p_warm[:, 0:P], lhsT=warm[:], rhs=warm[:],
                    start=True, stop=True, perf_mode=DR,
                )

        warm_mms(40)

        btile = const.tile([P, 5 * NNB], F32)
        xt8_sb = acts.tile([P, NKB, BL], F8)
        hxt8_sb = acts.tile([P, NKB, BL], F8)
        xtb_sb = acts.tile([P, NBF, BL], BF16)
        hxt32_sb = acts.tile([P, NKB, BL], F32)

        def w8_slab(m, nb):
            s = wp8.tile([P, NKB, P], F8, tag="w8slab", name=f"w8_{m}_{nb}")
            nc.sync.dma_start(s[:], w8[m, nb])
            return s

        def w16_slab(nb):
            s = wp16.tile([P, NBF, P], BF16, tag="w16slab", name=f"w16_{nb}")
            nc.sync.dma_start(s[:], w16[nb - BF0])
            return s

        def w8n_slab(nb):
            s = wp8n.tile([P, NF8, P], F8, tag="w8nslab", name=f"w8n_{nb}")
            nc.sync.dma_start(s[:], w8n[nb - BF0])
            return s

        def w8nh_slab(nb):
            s = wp8.tile([P, NKB, P], F8, tag="w8slab", name=f"w8nh_{nb}")
            nc.sync.dma_start(s[:], w8nh[nb])
            return s

        def w8nf_slab(nb):
            s = wp8.tile([P, NKB, P], F8, tag="w8slab", name=f"w8nf_{nb}")
            nc.sync.dma_start(s[:], w8nf[nb])
            return s

        def qdma(sb, dram, qi):
            nc.sync.dma_start(
                sb[:, 4 * qi : 4 * qi + 4, :], dram[:, 4 * qi : 4 * qi + 4, :]
            )

        def hx32dma(c0, c1):
            nc.sync.dma_start(hxt32_sb[:, c0:c1, :], hxt32[:, c0:c1, :])

        # Startup: serial need-order on the sync ring (startup is
        # HBM-bound; one ring in consumption order beats parallel rings).
        # Block 0 MM order is r-ih, z-ih, r-hh, z-hh, gi, gh.
        slabs = {}
        # The first act transfer rides the scalar ring: its ~1.5us queue
        # first-byte latency overlaps the sync ring's (which leads with
        # rih0), so the first real matmul's operands land in parallel.
        # Blocks 0-1 are fused xt-side-first: all five xt8 sweeps (~8.6us
        # of matmuls) run while hxt8 and the hh-side slabs stream in
        # behind them, so the hh sweeps start with their data landed.
        nc.scalar.dma_start(xt8_sb[:, 0:4, :], xt8[:, 0:4, :])
        nc.scalar.dma_start(btile[:], b[:])
        s0 = [None] * 4
        s1 = [None] * 4
        s0[0] = w8_slab(0, 0)
        qdma(xt8_sb, xt8, 1)
        qdma(xt8_sb, xt8, 2)
        s0[2] = w8_slab(2, 0)
        qdma(xt8_sb, xt8, 3)
        nf1 = w8nf_slab(1)
        s1[0] = w8_slab(0, 1)
        qdma(hxt8_sb, hxt8, 0)
        s1[2] = w8_slab(2, 1)
        qdma(hxt8_sb, hxt8, 1)
        s0[1] = w8_slab(1, 0)
        qdma(hxt8_sb, hxt8, 2)
        s0[3] = w8_slab(3, 0)
        qdma(hxt8_sb, hxt8, 3)
        slabs[0] = dict(s8=s0, s8nf=w8nf_slab(0), s8nh=w8nh_slab(0))
        hx32dma(0, 2)
        slabs[1] = dict(s8nf=nf1, s8=s1, s8nh=None)
        s1[1] = w8_slab(1, 1)
        s1[3] = w8_slab(3, 1)
        slabs[1]["s8nh"] = w8nh_slab(1)
        # block 2 MM order: r-ih, r-hh, gi, z-ih, gh, z-hh
        s2 = [None] * 4
        s2[0] = w8_slab(0, 2)
        s2[1] = w8_slab(1, 2)
        nf2 = w8nf_slab(2)
        s2[2] = w8_slab(2, 2)
        nh2 = w8nh_slab(2)
        s2[3] = w8_slab(3, 2)
        slabs[2] = dict(s8=s2, s8nf=nf2, s8nh=nh2)
        hx32dma(2, 4)

        def prefetch(m):
            # DMA in consumption order; hx32 blend chunks ride pairwise.
            s8 = [None] * 4
            if m < BF0:
                s8[0] = w8_slab(0, m)
                s8[1] = w8_slab(1, m)
                nf = w8nf_slab(m)
                s8[2] = w8_slab(2, m)
                nh = w8nh_slab(m)
                s8[3] = w8_slab(3, m)
                slabs[m] = dict(s8=s8, s8nf=nf, s8nh=nh)
            else:
                bf_first = (m % 2 == 0) or m == NNB - 1
                if m == BF0:
                    nc.sync.dma_start(xtb_sb[:], xtb[:])
                if bf_first:
                    s16 = w16_slab(m)
                    s8[0] = w8_slab(0, m)
                    s8[1] = w8_slab(1, m)
                    s8n = w8n_slab(m)
                    s8[2] = w8_slab(2, m)
                    s8nh = w8nh_slab(m)
                    s8[3] = w8_slab(3, m)
                else:
                    s8n = w8n_slab(m)
                    s8[0] = w8_slab(0, m)
                    s8[1] = w8_slab(1, m)
                    s8nh = w8nh_slab(m)
                    s8[2] = w8_slab(2, m)
                    s8[3] = w8_slab(3, m)
                    s16 = w16_slab(m)
                slabs[m] = dict(s8=s8, s16=s16, s8n=s8n, s8nh=s8nh)
            if m <= 8:
                hx32dma(2 * (m - 1), 2 * m)

        def mm_fp8(psum, slab, act_sb, start, stop):
            """8 DoubleRow matmuls sweeping all 16 k-chunks."""
            for j in range(NKB // 2):
                nc.tensor.matmul(
                    psum[:],
                    lhsT=slab[:, 2 * j : 2 * j + 2, :],
                    rhs=act_sb[:, 2 * j : 2 * j + 2, :],
                    start=(start and j == 0),
                    stop=(stop and j == NKB // 2 - 1),
                    perf_mode=DR,
                )

        def mm_n_bf(psum, s16, actb, start=True, stop=False):
            """n-gate ih half, bf16 segment (k-chunks 0..NBF-1)."""
            for k in range(NBF):
                nc.tensor.matmul(
                    psum[:],
                    lhsT=s16[:, k, :],
                    rhs=actb[:, k, :],
                    start=(start and k == 0),
                    stop=(stop and k == NBF - 1),
                )

        def mm_n_f8(psum, s8n, act8, start=False, stop=True):
            """n-gate ih half, fp8-DR segment (k-chunks NBF..15)."""
            for j in range(NF8 // 2):
                nc.tensor.matmul(
                    psum[:],
                    lhsT=s8n[:, 2 * j : 2 * j + 2, :],
                    rhs=act8[:, NBF + 2 * j : NBF + 2 * j + 2, :],
                    start=(start and j == 0),
                    stop=(stop and j == NF8 // 2 - 1),
                    perf_mode=DR,
                )

        def mm_fp8_cols(psum, slab, act_sb, c0, c1, start, stop):
            """DR sweep over all 16 k-chunks restricted to columns c0:c1."""
            for j in range(NKB // 2):
                nc.tensor.matmul(
                    psum[:, c0:c1],
                    lhsT=slab[:, 2 * j : 2 * j + 2, :],
                    rhs=act_sb[:, 2 * j : 2 * j + 2, c0:c1],
                    start=(start and j == 0),
                    stop=(stop and j == NKB // 2 - 1),
                    perf_mode=DR,
                )

        for nb in range(NNB):
            if 3 <= nb + 2 < NNB:
                prefetch(nb + 2)
            sl = slabs.pop(nb)
            s8 = sl["s8"]
            s8nh = sl["s8nh"]
            if nb < BF0:
                s8nf = sl["s8nf"]
            else:
                s16 = sl["s16"]
                s8n = sl["s8n"]

            if nb != 1:
                p_r = ps_r.tile([P, BL], F32)
                p_z = ps_z.tile([P, BL], F32)
                p_gi = ps_gi.tile([P, BL], F32)
                p_gh = ps_gh.tile([P, BL], F32)
            if nb == 0:
                # fused blocks 0-1, xt-side first (see startup comment)
                sl1 = slabs[1]
                p_r1 = ps_r.tile([P, BL], F32, tag="p_r", name="p_r1")
                p_z1 = ps_z.tile([P, BL], F32, tag="p_z", name="p_z1")
                p_gi1 = ps_gi.tile([P, BL], F32, tag="p_gi", name="p_gi1")
                p_gh1 = ps_gh.tile([P, BL], F32, tag="p_gh", name="p_gh1")
                sl1["psum"] = (p_r1, p_z1, p_gi1, p_gh1)
                mm_fp8(p_r, s8[0], xt8_sb, start=True, stop=False)
                mm_fp8(p_z, s8[2], xt8_sb, start=True, stop=False)
                mm_fp8(p_gi1, sl1["s8nf"], xt8_sb, start=True, stop=True)
                mm_fp8(p_r1, sl1["s8"][0], xt8_sb, start=True, stop=False)
                mm_fp8(p_z1, sl1["s8"][2], xt8_sb, start=True, stop=False)
                mm_fp8(p_r, s8[1], hxt8_sb, start=False, stop=True)
                mm_fp8(p_z, s8[3], hxt8_sb, start=False, stop=True)
                mm_fp8(p_gi, s8nf, xt8_sb, start=True, stop=True)
                mm_fp8(p_gh, s8nh, hxt8_sb, start=True, stop=True)
            elif nb == 1:
                p_r, p_z, p_gi, p_gh = sl["psum"]
                mm_fp8(p_r, s8[1], hxt8_sb, start=False, stop=True)
                mm_fp8(p_z, s8[3], hxt8_sb, start=False, stop=True)
                mm_fp8(p_gh, s8nh, hxt8_sb, start=True, stop=True)
            elif nb < BF0:
                mm_fp8(p_r, s8[0], xt8_sb, start=True, stop=False)
                mm_fp8(p_r, s8[1], hxt8_sb, start=False, stop=True)
                mm_fp8(p_gi, s8nf, xt8_sb, start=True, stop=True)
                mm_fp8(p_z, s8[2], xt8_sb, start=True, stop=False)
                mm_fp8(p_gh, s8nh, hxt8_sb, start=True, stop=True)
                mm_fp8(p_z, s8[3], hxt8_sb, start=False, stop=True)
            elif nb == NNB - 1:
                # last block: gi/r/gh early so the whole tanh chain runs
                # during the z sweeps; z in column halves into TWO PSUM
                # tiles so only the right half's chain trails the end.
                p_z2 = ps_z.tile([P, BL], F32, tag="p_z", name="p_z2")
                mm_fp8(p_r, s8[0], xt8_sb, start=True, stop=False)
                mm_fp8(p_r, s8[1], hxt8_sb, start=False, stop=True)
                mm_fp8(p_gh, s8nh, hxt8_sb, start=True, stop=True)
                mm_n_bf(p_gi, s16, xtb_sb)
                mm_n_f8(p_gi, s8n, xt8_sb)
                # asymmetric column split: the wide left part costs the
                # same matmul time as an even split, but the short right
                # part leaves only a 192-wide trailing chain.
                ZL = 320
                mm_fp8_cols(p_z, s8[2], xt8_sb, 0, ZL, start=True, stop=False)
                mm_fp8_cols(p_z, s8[3], hxt8_sb, 0, ZL, start=False, stop=True)
                mm_fp8_cols(p_z2, s8[2], xt8_sb, ZL, BL, start=True, stop=False)
                mm_fp8_cols(p_z2, s8[3], hxt8_sb, ZL, BL, start=False, stop=True)
            elif (nb % 2 == 0) or nb == NNB - 2:
                # bf16-first blocks (10, 12, 14): the bf16 segment joins
                # the previous block's bf16 tail so there is ~one
                # FWL<->DR switch per block.
                mm_n_bf(p_gi, s16, xtb_sb)
                mm_fp8(p_r, s8[0], xt8_sb, start=True, stop=False)
                mm_fp8(p_r, s8[1], hxt8_sb, start=False, stop=True)
                mm_n_f8(p_gi, s8n, xt8_sb)
                mm_fp8(p_z, s8[2], xt8_sb, start=True, stop=False)
                mm_fp8(p_gh, s8nh, hxt8_sb, start=True, stop=True)
                mm_fp8(p_z, s8[3], hxt8_sb, start=False, stop=True)
            else:
                # bf16-last blocks (9, 11, 13)
                mm_n_f8(p_gi, s8n, xt8_sb, start=True, stop=False)
                mm_fp8(p_r, s8[0], xt8_sb, start=True, stop=False)
                mm_fp8(p_r, s8[1], hxt8_sb, start=False, stop=True)
                mm_fp8(p_gh, s8nh, hxt8_sb, start=True, stop=True)
                mm_fp8(p_z, s8[2], xt8_sb, start=True, stop=False)
                mm_fp8(p_z, s8[3], hxt8_sb, start=False, stop=True)
                mm_n_bf(p_gi, s16, xtb_sb, start=False, stop=True)

            def bias_ap(g):
                return btile[:, g * NNB + nb : g * NNB + nb + 1]

            if nb == NNB - 1:
                # z-last tail: r/t/x/tanh/d run during the z sweeps; after
                # the final (right-half) z matmul only sigmoid/mul/add/DMA
                # for that half trail, in quarters.
                r_sb = gates.tile([P, BL], F32, tag="r")
                nc.scalar.activation(
                    r_sb[:], p_r[:], mybir.ActivationFunctionType.Sigmoid,
                    bias=bias_ap(0), scale=1.0 / sw,
                )
                t_sb = gates.tile([P, BL], F32, tag="t")
                nc.vector.scalar_tensor_tensor(
                    t_sb[:], p_gh[:], bias_ap(3), r_sb[:],
                    op0=mybir.AluOpType.add, op1=mybir.AluOpType.mult,
                )
                x_sb = gates.tile([P, BL], F32, tag="x")
                n_sb = gates.tile([P, BL], F32, tag="n")
                d_sb = gates.tile([P, BL], F32, tag="d")
                QH = BL // 4
                for q in range(4):
                    qs = slice(q * QH, (q + 1) * QH)
                    nc.vector.tensor_add(x_sb[:, qs], t_sb[:, qs], p_gi[:, qs])
                    nc.scalar.activation(
                        n_sb[:, qs], x_sb[:, qs],
                        mybir.ActivationFunctionType.Tanh,
                        bias=bias_ap(2), scale=1.0 / sw,
                    )
                    nc.vector.tensor_sub(
                        d_sb[:, qs], hxt32_sb[:, nb, qs], n_sb[:, qs]
                    )
                z_sb = gates.tile([P, BL], F32, tag="z")
                e_sb = gates.tile([P, BL], F32, tag="e")
                o_sb = opool.tile([P, BL], F32, tag="o")
                for qs, pz, ring, eng in (
                    (slice(0, ZL), p_z, nc.sync, nc.vector),
                    (slice(ZL, BL), p_z2, nc.scalar, nc.vector),
                ):
                    nc.scalar.activation(
                        z_sb[:, qs], pz[:, qs],
                        mybir.ActivationFunctionType.Sigmoid,
                        bias=bias_ap(1), scale=1.0 / sw,
                    )
                    eng.tensor_mul(e_sb[:, qs], z_sb[:, qs], d_sb[:, qs])
                    eng.tensor_add(o_sb[:, qs], n_sb[:, qs], e_sb[:, qs])
                    ring.dma_start(out[nb * P : (nb + 1) * P, qs], o_sb[:, qs])
                continue

            # r = sigmoid(p_r/SW + b_ih0 + b_hh0)
            r_sb = gates.tile([P, BL], F32, tag="r")
            nc.scalar.activation(
                r_sb[:], p_r[:], mybir.ActivationFunctionType.Sigmoid,
                bias=bias_ap(0), scale=1.0 / sw,
            )
            # tanh chain emitted BEFORE the z sigmoid (program order per
            # engine; keeps scalar free for the last block's tanh).
            t_sb = gates.tile([P, BL], F32, tag="t")
            nc.vector.scalar_tensor_tensor(
                t_sb[:], p_gh[:], bias_ap(3), r_sb[:],
                op0=mybir.AluOpType.add, op1=mybir.AluOpType.mult,
            )
            x_sb = gates.tile([P, BL], F32, tag="x")
            nc.vector.tensor_add(x_sb[:], t_sb[:], p_gi[:])
            n_sb = gates.tile([P, BL], F32, tag="n")
            nc.scalar.activation(
                n_sb[:], x_sb[:], mybir.ActivationFunctionType.Tanh,
                bias=bias_ap(2), scale=1.0 / sw,
            )
            # d = hx - n;  hx from the fp32 act copy
            d_sb = gates.tile([P, BL], F32, tag="d")
            nc.vector.tensor_sub(d_sb[:], hxt32_sb[:, nb, :], n_sb[:])
            # z = sigmoid(p_z/SW + b_ih1 + b_hh1), then out = n + z*d
            z_sb = gates.tile([P, BL], F32, tag="z")
            e_sb = gates.tile([P, BL], F32, tag="e")
            o_sb = opool.tile([P, BL], F32, tag="o")
            nc.scalar.activation(
                z_sb[:], p_z[:], mybir.ActivationFunctionType.Sigmoid,
                bias=bias_ap(1), scale=1.0 / sw,
            )
            # the two blocks before last run e/o on gpsimd so the vector
            # queue is free for the last block's tanh chain
            eng = nc.gpsimd if nb >= NNB - 3 else nc.vector
            eng.tensor_mul(e_sb[:], z_sb[:], d_sb[:])
            eng.tensor_add(o_sb[:], n_sb[:], e_sb[:])
            nc.gpsimd.dma_start(out[nb * P : (nb + 1) * P, :], o_sb[:])

    nc.compile()
    return nc


def _pack_inputs(input, hx, weight_ih, weight_hh, bias_ih, bias_hh, sw):
    """Host-side shard + layout packing. Returns per-core input maps."""
    input = np.ascontiguousarray(np.asarray(input, dtype=np.float32))
    hx = np.ascontiguousarray(np.asarray(hx, dtype=np.float32))
    weight_ih = np.asarray(weight_ih, dtype=np.float32)
    weight_hh = np.asarray(weight_hh, dtype=np.float32)
    bias_ih = np.asarray(bias_ih, dtype=np.float32)
    bias_hh = np.asarray(bias_hh, dtype=np.float32)

    # wpack[m, nb, kp, k, n] = W_m[k*128+kp, nb*128+n]
    def wpack(mats, scale, dt):
        return np.ascontiguousarray(
            np.stack(
                [
                    np.asarray(wm * scale, dtype=dt)
                    .reshape(NKB, P, NNB, P)
                    .transpose(2, 1, 0, 3)
                    for wm in mats
                ]
            )
        )

    w8p = wpack(
        [weight_ih[0], weight_hh[0], weight_ih[1], weight_hh[1]],
        sw, ml_dtypes.float8_e4m3,
    )
    wn_f = wpack([weight_ih[2], weight_hh[2]], sw, np.float32)
    w16p = np.ascontiguousarray(
        wn_f[0, BF0:, :, :NBF, :].astype(ml_dtypes.bfloat16)
    )
    w8np = np.ascontiguousarray(
        wn_f[0, BF0:, :, NBF:, :].astype(ml_dtypes.float8_e4m3)
    )
    w8nhp = np.ascontiguousarray(wn_f[1].astype(ml_dtypes.float8_e4m3))
    w8nfp = np.ascontiguousarray(wn_f[0, :BF0].astype(ml_dtypes.float8_e4m3))

    # bpack[p, g*16+nb] = bias_g[nb*128+p]
    # g order: r_sum, z_sum, ih2, hh2, neg_z_sum (unused).  hh2 is xSW
    # because it adds to the SW-scaled PSUM before the tanh descale.
    bias_all = np.stack(
        [bias_ih[0] + bias_hh[0], bias_ih[1] + bias_hh[1], bias_ih[2],
         np.float32(sw) * bias_hh[2], -(bias_ih[1] + bias_hh[1])]
    )  # [5, H]
    bpack = np.ascontiguousarray(
        bias_all.reshape(5, NNB, P).transpose(2, 0, 1).reshape(P, 5 * NNB)
    )

    def t_pack(a, dt):
        # [BL, H] -> [P, NKB, BL] with [kp, k, m] = a[m, k*128+kp]
        return np.ascontiguousarray(
            a.T.reshape(NKB, P, BL).transpose(1, 0, 2).astype(dt)
        )

    in_maps = []
    for c in range(N_CORES):
        sl = slice(c * BL, (c + 1) * BL)
        in_maps.append(
            {
                "xt8": t_pack(input[sl], ml_dtypes.float8_e4m3),
                "hxt8": t_pack(hx[sl], ml_dtypes.float8_e4m3),
                "xtb": np.ascontiguousarray(
                    t_pack(input[sl], ml_dtypes.bfloat16)[:, :NBF, :]
                ),
                "hxt32": t_pack(hx[sl], np.float32),
                "w8": w8p,
                "w16": w16p,
                "w8n": w8np,
                "w8nh": w8nhp,
                "w8nf": w8nfp,
                "b": bpack,
            }
        )
    return in_maps


_PROGRAM_CACHE = {}


def kernel(input, hx, weight_ih, weight_hh, bias_ih, bias_hh, _trace=False):
    wmax = float(
        max(
            np.abs(np.asarray(weight_ih, dtype=np.float32)).max(),
            np.abs(np.asarray(weight_hh, dtype=np.float32)).max(),
        )
    )
    sw = SWMAX / wmax if wmax > 0 else 64.0
    key = round(sw, 6)
    if key not in _PROGRAM_CACHE:
        _PROGRAM_CACHE[key] = _build_program(sw)
    nc = _PROGRAM_CACHE[key]
    in_maps = _pack_inputs(input, hx, weight_ih, weight_hh, bias_ih, bias_hh, sw)
    res = run_bass_kernel_spmd(nc, in_maps, list(range(N_CORES)), trace=_trace)
    out = np.empty((B, H), dtype=np.float32)
    for c in range(N_CORES):
        out[c * BL : (c + 1) * BL] = res.results[c]["out"].T
    if _trace:
        kernel.last_exec_time_ns = res.exec_time_ns
    return out


# revision 31
# speedup vs baseline: 1.0039x; 1.0039x over previous
"""GRU cell (B=4096, H=2048) on 8 TRN2 NeuronCores — fp8/bf16 mixed.

Sharding: data-parallel over batch — each core computes 512 rows; weights
replicated, no collectives.

Per-core compute in transposed space (hidden on partitions, batch free).
All weights are pre-scaled on the host so max|w8| = 0.9995: the weights
are uniform(+-stdv), and placing the max just under a binade boundary
fills e4m3's finest relative grid (quantization MSE x0.73 vs a scale
that lands max|w8| mid-binade).  Every activation descales with 1/SW.

Precision schedule: r/z gates and the n-gate hh half are fully fp8-e4m3
DoubleRow (2 k-chunks per matmul; the hh error is attenuated by r inside
tanh(gi2 + r*gh2)).  The n-gate ih half is all-fp8 for hidden blocks
0-10 and 8 bf16 k-chunks + 8 fp8-DR chunks for blocks 11-15 (error
variance is linear in the bf16 chunk count, so concentrating the budget
in few blocks halves the FWL<->DoubleRow mode switches and keeps bf16
out of the HBM-bound startup window).  The hx used in the output blend
is fp32 (error margin at DMA-only cost).

Startup: the DMA engines ramp (~110 GB/s for the first ~5us under
8-core HBM contention, ~350 GB/s after), so block 0's operands land
~7us after the first transfer is issued.  A tiny memset tile feeds
FD=128 fp8-DR warm-up matmuls that bridge that whole window — idling
the PE >3.4us would reset the HAM clock ramp back to 1.2 GHz.  Blocks
0-1 are fused xt-side-first: all five xt8 sweeps run while hxt8 and
the hh-side slabs stream in behind them, so the hh sweeps start with
their data landed.  Weight slabs prefetch ~2 blocks ahead of use.

Last block: r/gh/gi early so the whole tanh chain runs during the z
sweeps; z is computed in 320/192 column parts into TWO PSUM tiles (PSUM
read deps are whole-tile) so the wide part's sigmoid/mul/add/DMA runs
under the short part's matmuls and only a 192-wide chain trails the
final matmul.  The first act transfer rides the scalar ring so its
queue first-byte latency overlaps the sync ring's.

Measured on HW: 190.7us, rel err 1.968e-2 (gate 2e-2); the numpy
emulation of the quantization error predicts the HW rel err to ~3-4
digits (emu 1.9684e-2).
"""

from contextlib import ExitStack

import ml_dtypes
import numpy as np

import concourse.bass as bass
import concourse.tile as tile
from concourse import bacc, mybir
from concourse.bass_utils import run_bass_kernel_spmd

H = 2048
B = 4096
N_CORES = 8
BL = B // N_CORES  # 512 batch rows per core
P = 128
NKB = H // P  # 16 contraction chunks
NNB = H // P  # 16 hidden (output) blocks
F32 = mybir.dt.float32
F8 = mybir.dt.float8e4
BF16 = mybir.dt.bfloat16
DR = mybir.MatmulPerfMode.DoubleRow
NBF = 8  # n-gate ih-half bf16 k-chunks (blocks BF0..15; rest fp8-DR)
NF8 = NKB - NBF
BF0 = 11  # first hidden block with the bf16 segment
NFP = NNB - BF0  # number of bf16-carrying blocks
SWMAX = 0.9995  # target max|w8| — just under the binade boundary

# w8 matrix order: 0 r_ih, 1 r_hh, 2 z_ih, 3 z_hh.  w8nf: full-K fp8
# n-ih for blocks 0..BF0-1.  w16/w8n: the n-ih split for blocks BF0+.
# w8nh: n-gate hh half, fully fp8.  b_hh2 is pre-scaled xSW so it can
# add to the SW-scaled PSUM before the tanh descale.


def _build_program(sw: float) -> bacc.Bacc:
    nc = bacc.Bacc(
        "TRN2", target_bir_lowering=False, debug=False, num_devices=N_CORES
    )

    xt8 = nc.dram_tensor("xt8", [P, NKB, BL], F8, kind="ExternalInput").ap()
    hxt8 = nc.dram_tensor("hxt8", [P, NKB, BL], F8, kind="ExternalInput").ap()
    xtb = nc.dram_tensor("xtb", [P, NBF, BL], BF16, kind="ExternalInput").ap()
    hxt32 = nc.dram_tensor("hxt32", [P, NKB, BL], F32, kind="ExternalInput").ap()
    w8 = nc.dram_tensor("w8", [4, NNB, P, NKB, P], F8, kind="ExternalInput").ap()
    w16 = nc.dram_tensor("w16", [NFP, P, NBF, P], BF16, kind="ExternalInput").ap()
    w8n = nc.dram_tensor("w8n", [NFP, P, NF8, P], F8, kind="ExternalInput").ap()
    w8nh = nc.dram_tensor("w8nh", [NNB, P, NKB, P], F8, kind="ExternalInput").ap()
    w8nf = nc.dram_tensor("w8nf", [BF0, P, NKB, P], F8, kind="ExternalInput").ap()
    b = nc.dram_tensor("b", [P, 5 * NNB], F32, kind="ExternalInput").ap()
    out = nc.dram_tensor("out", [H, BL], F32, kind="ExternalOutput").ap()

    with tile.TileContext(nc) as tc, ExitStack() as ctx:
        const = ctx.enter_context(tc.tile_pool(name="const", bufs=1))
        acts = ctx.enter_context(tc.tile_pool(name="acts", bufs=1))
        wp8 = ctx.enter_context(tc.tile_pool(name="wp8", bufs=24))
        wp8n = ctx.enter_context(tc.tile_pool(name="wp8n", bufs=4))
        wp16 = ctx.enter_context(tc.tile_pool(name="wp16", bufs=4))
        gates = ctx.enter_context(tc.tile_pool(name="gates", bufs=2))
        opool = ctx.enter_context(tc.tile_pool(name="opool", bufs=3))
        ps_r = ctx.enter_context(tc.tile_pool(name="ps_r", bufs=2, space="PSUM"))
        ps_z = ctx.enter_context(tc.tile_pool(name="ps_z", bufs=2, space="PSUM"))
        ps_gi = ctx.enter_context(tc.tile_pool(name="ps_gi", bufs=2, space="PSUM"))
        ps_gh = ctx.enter_context(tc.tile_pool(name="ps_gh", bufs=2, space="PSUM"))

        # PE warm-up bridge (see module docstring).
        warm = const.tile([P, 2, P], F8)
        nc.gpsimd.memset(warm[:], 0.0)
        p_warm = ps_gh.tile([P, BL], F32, tag="p_gh", name="p_warm")

        def warm_mms(n):
            for _ in range(n):
                nc.tensor.matmul(
                    p_warm[:, 0:P], lhsT=warm[:], rhs=warm[:],
                    start=True, stop=True, perf_mode=DR,
                )

        warm_mms(40)

        btile = const.tile([P, 5 * NNB], F32)
        xt8_sb = acts.tile([P, NKB, BL], F8)
        hxt8_sb = acts.tile([P, NKB, BL], F8)
        xtb_sb = acts.tile([P, NBF, BL], BF16)
        hxt32_sb = acts.tile([P, NKB, BL], F32)

        def w8_slab(m, nb):
            s = wp8.tile([P, NKB, P], F8, tag="w8slab", name=f"w8_{m}_{nb}")
            nc.sync.dma_start(s[:], w8[m, nb])
            return s

        def w16_slab(nb):
            s = wp16.tile([P, NBF, P], BF16, tag="w16slab", name=f"w16_{nb}")
            nc.sync.dma_start(s[:], w16[nb - BF0])
            return s

        def w8n_slab(nb):
            s = wp8n.tile([P, NF8, P], F8, tag="w8nslab", name=f"w8n_{nb}")
            nc.sync.dma_start(s[:], w8n[nb - BF0])
            return s

        def w8nh_slab(nb):
            s = wp8.tile([P, NKB, P], F8, tag="w8slab", name=f"w8nh_{nb}")
            nc.sync.dma_start(s[:], w8nh[nb])
            return s

        def w8nf_slab(nb):
            s = wp8.tile([P, NKB, P], F8, tag="w8slab", name=f"w8nf_{nb}")
            nc.sync.dma_start(s[:], w8nf[nb])
            return s

        def qdma(sb, dram, qi):
            nc.sync.dma_start(
                sb[:, 4 * qi : 4 * qi + 4, :], dram[:, 4 * qi : 4 * qi + 4, :]
            )

        def hx32dma(c0, c1):
            nc.sync.dma_start(hxt32_sb[:, c0:c1, :], hxt32[:, c0:c1, :])

        # Startup: serial need-order on the sync ring (startup is
        # HBM-bound; one ring in consumption order beats parallel rings).
        # Block 0 MM order is r-ih, z-ih, r-hh, z-hh, gi, gh.
        slabs = {}
        # The first act transfer rides the scalar ring: its ~1.5us queue
        # first-byte latency overlaps the sync ring's (which leads with
        # rih0), so the first real matmul's operands land in parallel.
        # Blocks 0-1 are fused xt-side-first: all five xt8 sweeps (~8.6us
        # of matmuls) run while hxt8 and the hh-side slabs stream in
        # behind them, so the hh sweeps start with their data landed.
        nc.scalar.dma_start(xt8_sb[:, 0:4, :], xt8[:, 0:4, :])
        nc.scalar.dma_start(xt8_sb[:, 4:8, :], xt8[:, 4:8, :])
        nc.scalar.dma_start(btile[:], b[:])
        s0 = [None] * 4
        s1 = [None] * 4
        s0[0] = w8_slab(0, 0)
        qdma(xt8_sb, xt8, 2)
        s0[2] = w8_slab(2, 0)
        qdma(xt8_sb, xt8, 3)
        nf1 = w8nf_slab(1)
        s1[0] = w8_slab(0, 1)
        qdma(hxt8_sb, hxt8, 0)
        s1[2] = w8_slab(2, 1)
        qdma(hxt8_sb, hxt8, 1)
        s0[1] = w8_slab(1, 0)
        qdma(hxt8_sb, hxt8, 2)
        s0[3] = w8_slab(3, 0)
        qdma(hxt8_sb, hxt8, 3)
        slabs[0] = dict(s8=s0, s8nf=w8nf_slab(0), s8nh=w8nh_slab(0))
        hx32dma(0, 2)
        slabs[1] = dict(s8nf=nf1, s8=s1, s8nh=None)
        s1[1] = w8_slab(1, 1)
        s1[3] = w8_slab(3, 1)
        slabs[1]["s8nh"] = w8nh_slab(1)
        # block 2 MM order: r-ih, r-hh, gi, z-ih, gh, z-hh
        s2 = [None] * 4
        s2[0] = w8_slab(0, 2)
        s2[1] = w8_slab(1, 2)
        nf2 = w8nf_slab(2)
        s2[2] = w8_slab(2, 2)
        nh2 = w8nh_slab(2)
        s2[3] = w8_slab(3, 2)
        slabs[2] = dict(s8=s2, s8nf=nf2, s8nh=nh2)
        hx32dma(2, 4)

        def prefetch(m):
            # DMA in consumption order; hx32 blend chunks ride pairwise.
            s8 = [None] * 4
            if m < BF0:
                s8[0] = w8_slab(0, m)
                s8[1] = w8_slab(1, m)
                nf = w8nf_slab(m)
                s8[2] = w8_slab(2, m)
                nh = w8nh_slab(m)
                s8[3] = w8_slab(3, m)
                slabs[m] = dict(s8=s8, s8nf=nf, s8nh=nh)
            else:
                bf_first = (m % 2 == 0) or m == NNB - 1
                if m == BF0:
                    nc.sync.dma_start(xtb_sb[:], xtb[:])
                if bf_first:
                    s16 = w16_slab(m)
                    s8[0] = w8_slab(0, m)
                    s8[1] = w8_slab(1, m)
                    s8n = w8n_slab(m)
                    s8[2] = w8_slab(2, m)
                    s8nh = w8nh_slab(m)
                    s8[3] = w8_slab(3, m)
                else:
                    s8n = w8n_slab(m)
                    s8[0] = w8_slab(0, m)
                    s8[1] = w8_slab(1, m)
                    s8nh = w8nh_slab(m)
                    s8[2] = w8_slab(2, m)
                    s8[3] = w8_slab(3, m)
                    s16 = w16_slab(m)
                slabs[m] = dict(s8=s8, s16=s16, s8n=s8n, s8nh=s8nh)
            if m <= 8:
                hx32dma(2 * (m - 1), 2 * m)

        def mm_fp8(psum, slab, act_sb, start, stop):
            """8 DoubleRow matmuls sweeping all 16 k-chunks."""
            for j in range(NKB // 2):
                nc.tensor.matmul(
                    psum[:],
                    lhsT=slab[:, 2 * j : 2 * j + 2, :],
                    rhs=act_sb[:, 2 * j : 2 * j + 2, :],
                    start=(start and j == 0),
                    stop=(stop and j == NKB // 2 - 1),
                    perf_mode=DR,
                )

        def mm_n_bf(psum, s16, actb, start=True, stop=False):
            """n-gate ih half, bf16 segment (k-chunks 0..NBF-1)."""
            for k in range(NBF):
                nc.tensor.matmul(
                    psum[:],
                    lhsT=s16[:, k, :],
                    rhs=actb[:, k, :],
                    start=(start and k == 0),
                    stop=(stop and k == NBF - 1),
                )

        def mm_n_f8(psum, s8n, act8, start=False, stop=True):
            """n-gate ih half, fp8-DR segment (k-chunks NBF..15)."""
            for j in range(NF8 // 2):
                nc.tensor.matmul(
                    psum[:],
                    lhsT=s8n[:, 2 * j : 2 * j + 2, :],
                    rhs=act8[:, NBF + 2 * j : NBF + 2 * j + 2, :],
                    start=(start and j == 0),
                    stop=(stop and j == NF8 // 2 - 1),
                    perf_mode=DR,
                )

        def mm_fp8_cols(psum, slab, act_sb, c0, c1, start, stop):
            """DR sweep over all 16 k-chunks restricted to columns c0:c1."""
            for j in range(NKB // 2):
                nc.tensor.matmul(
                    psum[:, c0:c1],
                    lhsT=slab[:, 2 * j : 2 * j + 2, :],
                    rhs=act_sb[:, 2 * j : 2 * j + 2, c0:c1],
                    start=(start and j == 0),
                    stop=(stop and j == NKB // 2 - 1),
                    perf_mode=DR,
                )

        for nb in range(NNB):
            if 3 <= nb + 2 < NNB:
                prefetch(nb + 2)
            sl = slabs.pop(nb)
            s8 = sl["s8"]
            s8nh = sl["s8nh"]
            if nb < BF0:
                s8nf = sl["s8nf"]
            else:
                s16 = sl["s16"]
                s8n = sl["s8n"]

            if nb != 1:
                p_r = ps_r.tile([P, BL], F32)
                p_z = ps_z.tile([P, BL], F32)
                p_gi = ps_gi.tile([P, BL], F32)
                p_gh = ps_gh.tile([P, BL], F32)
            if nb == 0:
                # fused blocks 0-1, xt-side first (see startup comment)
                sl1 = slabs[1]
                p_r1 = ps_r.tile([P, BL], F32, tag="p_r", name="p_r1")
                p_z1 = ps_z.tile([P, BL], F32, tag="p_z", name="p_z1")
                p_gi1 = ps_gi.tile([P, BL], F32, tag="p_gi", name="p_gi1")
                p_gh1 = ps_gh.tile([P, BL], F32, tag="p_gh", name="p_gh1")
                sl1["psum"] = (p_r1, p_z1, p_gi1, p_gh1)
                mm_fp8(p_r, s8[0], xt8_sb, start=True, stop=False)
                mm_fp8(p_z, s8[2], xt8_sb, start=True, stop=False)
                mm_fp8(p_gi1, sl1["s8nf"], xt8_sb, start=True, stop=True)
                mm_fp8(p_r1, sl1["s8"][0], xt8_sb, start=True, stop=False)
                mm_fp8(p_z1, sl1["s8"][2], xt8_sb, start=True, stop=False)
                mm_fp8(p_r, s8[1], hxt8_sb, start=False, stop=True)
                mm_fp8(p_z, s8[3], hxt8_sb, start=False, stop=True)
                mm_fp8(p_gi, s8nf, xt8_sb, start=True, stop=True)
                mm_fp8(p_gh, s8nh, hxt8_sb, start=True, stop=True)
            elif nb == 1:
                p_r, p_z, p_gi, p_gh = sl["psum"]
                mm_fp8(p_r, s8[1], hxt8_sb, start=False, stop=True)
                mm_fp8(p_z, s8[3], hxt8_sb, start=False, stop=True)
                mm_fp8(p_gh, s8nh, hxt8_sb, start=True, stop=True)
            elif nb < BF0:
                mm_fp8(p_r, s8[0], xt8_sb, start=True, stop=False)
                mm_fp8(p_r, s8[1], hxt8_sb, start=False, stop=True)
                mm_fp8(p_gi, s8nf, xt8_sb, start=True, stop=True)
                mm_fp8(p_z, s8[2], xt8_sb, start=True, stop=False)
                mm_fp8(p_gh, s8nh, hxt8_sb, start=True, stop=True)
                mm_fp8(p_z, s8[3], hxt8_sb, start=False, stop=True)
            elif nb == NNB - 1:
                # last block: gi/r/gh early so the whole tanh chain runs
                # during the z sweeps; z in column halves into TWO PSUM
                # tiles so only the right half's chain trails the end.
                p_z2 = ps_z.tile([P, BL], F32, tag="p_z", name="p_z2")
                mm_fp8(p_r, s8[0], xt8_sb, start=True, stop=False)
                mm_fp8(p_r, s8[1], hxt8_sb, start=False, stop=True)
                mm_fp8(p_gh, s8nh, hxt8_sb, start=True, stop=True)
                mm_n_bf(p_gi, s16, xtb_sb)
                mm_n_f8(p_gi, s8n, xt8_sb)
                # asymmetric column split: the wide left part costs the
                # same matmul time as an even split, but the short right
                # part leaves only a 192-wide trailing chain.
                ZL = 320
                mm_fp8_cols(p_z, s8[2], xt8_sb, 0, ZL, start=True, stop=False)
                mm_fp8_cols(p_z, s8[3], hxt8_sb, 0, ZL, start=False, stop=True)
                mm_fp8_cols(p_z2, s8[2], xt8_sb, ZL, BL, start=True, stop=False)
                mm_fp8_cols(p_z2, s8[3], hxt8_sb, ZL, BL, start=False, stop=True)
            elif (nb % 2 == 0) or nb == NNB - 2:
                # bf16-first blocks (10, 12, 14): the bf16 segment joins
                # the previous block's bf16 tail so there is ~one
                # FWL<->DR switch per block.
                mm_n_bf(p_gi, s16, xtb_sb)
                mm_fp8(p_r, s8[0], xt8_sb, start=True, stop=False)
                mm_fp8(p_r, s8[1], hxt8_sb, start=False, stop=True)
                mm_n_f8(p_gi, s8n, xt8_sb)
                mm_fp8(p_z, s8[2], xt8_sb, start=True, stop=False)
                mm_fp8(p_gh, s8nh, hxt8_sb, start=True, stop=True)
                mm_fp8(p_z, s8[3], hxt8_sb, start=False, stop=True)
            else:
                # bf16-last blocks (9, 11, 13)
                mm_n_f8(p_gi, s8n, xt8_sb, start=True, stop=False)
                mm_fp8(p_r, s8[0], xt8_sb, start=True, stop=False)
                mm_fp8(p_r, s8[1], hxt8_sb, start=False, stop=True)
                mm_fp8(p_gh, s8nh, hxt8_sb, start=True, stop=True)
                mm_fp8(p_z, s8[2], xt8_sb, start=True, stop=False)
                mm_fp8(p_z, s8[3], hxt8_sb, start=False, stop=True)
                mm_n_bf(p_gi, s16, xtb_sb, start=False, stop=True)

            def bias_ap(g):
                return btile[:, g * NNB + nb : g * NNB + nb + 1]

            if nb == NNB - 1:
                # z-last tail: r/t/x/tanh/d run during the z sweeps; after
                # the final (right-half) z matmul only sigmoid/mul/add/DMA
                # for that half trail, in quarters.
                r_sb = gates.tile([P, BL], F32, tag="r")
                nc.scalar.activation(
                    r_sb[:], p_r[:], mybir.ActivationFunctionType.Sigmoid,
                    bias=bias_ap(0), scale=1.0 / sw,
                )
                t_sb = gates.tile([P, BL], F32, tag="t")
                nc.vector.scalar_tensor_tensor(
                    t_sb[:], p_gh[:], bias_ap(3), r_sb[:],
                    op0=mybir.AluOpType.add, op1=mybir.AluOpType.mult,
                )
                x_sb = gates.tile([P, BL], F32, tag="x")
                n_sb = gates.tile([P, BL], F32, tag="n")
                d_sb = gates.tile([P, BL], F32, tag="d")
                QH = BL // 4
                for q in range(4):
                    qs = slice(q * QH, (q + 1) * QH)
                    nc.vector.tensor_add(x_sb[:, qs], t_sb[:, qs], p_gi[:, qs])
                    nc.scalar.activation(
                        n_sb[:, qs], x_sb[:, qs],
                        mybir.ActivationFunctionType.Tanh,
                        bias=bias_ap(2), scale=1.0 / sw,
                    )
                    nc.vector.tensor_sub(
                        d_sb[:, qs], hxt32_sb[:, nb, qs], n_sb[:, qs]
                    )
                z_sb = gates.tile([P, BL], F32, tag="z")
                e_sb = gates.tile([P, BL], F32, tag="e")
                o_sb = opool.tile([P, BL], F32, tag="o")
                for qs, pz, ring, eng in (
                    (slice(0, ZL), p_z, nc.sync, nc.vector),
                    (slice(ZL, BL), p_z2, nc.scalar, nc.vector),
                ):
                    nc.scalar.activation(
                        z_sb[:, qs], pz[:, qs],
                        mybir.ActivationFunctionType.Sigmoid,
                        bias=bias_ap(1), scale=1.0 / sw,
                    )
                    eng.tensor_mul(e_sb[:, qs], z_sb[:, qs], d_sb[:, qs])
                    eng.tensor_add(o_sb[:, qs], n_sb[:, qs], e_sb[:, qs])
                    ring.dma_start(out[nb * P : (nb + 1) * P, qs], o_sb[:, qs])
                continue

            # r = sigmoid(p_r/SW + b_ih0 + b_hh0)
            r_sb = gates.tile([P, BL], F32, tag="r")
            nc.scalar.activation(
                r_sb[:], p_r[:], mybir.ActivationFunctionType.Sigmoid,
                bias=bias_ap(0), scale=1.0 / sw,
            )
            # tanh chain emitted BEFORE the z sigmoid (program order per
            # engine; keeps scalar free for the last block's tanh).
            t_sb = gates.tile([P, BL], F32, tag="t")
            nc.vector.scalar_tensor_tensor(
                t_sb[:], p_gh[:], bias_ap(3), r_sb[:],
                op0=mybir.AluOpType.add, op1=mybir.AluOpType.mult,
            )
            x_sb = gates.tile([P, BL], F32, tag="x")
            nc.vector.tensor_add(x_sb[:], t_sb[:], p_gi[:])
            n_sb = gates.tile([P, BL], F32, tag="n")
            nc.scalar.activation(
                n_sb[:], x_sb[:], mybir.ActivationFunctionType.Tanh,
                bias=bias_ap(2), scale=1.0 / sw,
            )
            # d = hx - n;  hx from the fp32 act copy
            d_sb = gates.tile([P, BL], F32, tag="d")
            nc.vector.tensor_sub(d_sb[:], hxt32_sb[:, nb, :], n_sb[:])
            # z = sigmoid(p_z/SW + b_ih1 + b_hh1), then out = n + z*d
            z_sb = gates.tile([P, BL], F32, tag="z")
            e_sb = gates.tile([P, BL], F32, tag="e")
            o_sb = opool.tile([P, BL], F32, tag="o")
            nc.scalar.activation(
                z_sb[:], p_z[:], mybir.ActivationFunctionType.Sigmoid,
                bias=bias_ap(1), scale=1.0 / sw,
            )
            # the two blocks before last run e/o on gpsimd so the vector
            # queue is free for the last block's tanh chain
            eng = nc.gpsimd if nb >= NNB - 3 else nc.vector
            eng.tensor_mul(e_sb[:], z_sb[:], d_sb[:])
            eng.tensor_add(o_sb[:], n_sb[:], e_sb[:])
            nc.gpsimd.dma_start(out[nb * P : (nb + 1) * P, :], o_sb[:])

    nc.compile()
    return nc


def _pack_inputs(input, hx, weight_ih, weight_hh, bias_ih, bias_hh, sw):
    """Host-side shard + layout packing. Returns per-core input maps."""
    input = np.ascontiguousarray(np.asarray(input, dtype=np.float32))
    hx = np.ascontiguousarray(np.asarray(hx, dtype=np.float32))
    weight_ih = np.asarray(weight_ih, dtype=np.float32)
    weight_hh = np.asarray(weight_hh, dtype=np.float32)
    bias_ih = np.asarray(bias_ih, dtype=np.float32)
    bias_hh = np.asarray(bias_hh, dtype=np.float32)

    # wpack[m, nb, kp, k, n] = W_m[k*128+kp, nb*128+n]
    def wpack(mats, scale, dt):
        return np.ascontiguousarray(
            np.stack(
                [
                    np.asarray(wm * scale, dtype=dt)
                    .reshape(NKB, P, NNB, P)
                    .transpose(2, 1, 0, 3)
                    for wm in mats
                ]
            )
        )

    w8p = wpack(
        [weight_ih[0], weight_hh[0], weight_ih[1], weight_hh[1]],
        sw, ml_dtypes.float8_e4m3,
    )
    wn_f = wpack([weight_ih[2], weight_hh[2]], sw, np.float32)
    w16p = np.ascontiguousarray(
        wn_f[0, BF0:, :, :NBF, :].astype(ml_dtypes.bfloat16)
    )
    w8np = np.ascontiguousarray(
        wn_f[0, BF0:, :, NBF:, :].astype(ml_dtypes.float8_e4m3)
    )
    w8nhp = np.ascontiguousarray(wn_f[1].astype(ml_dtypes.float8_e4m3))
    w8nfp = np.ascontiguousarray(wn_f[0, :BF0].astype(ml_dtypes.float8_e4m3))

    # bpack[p, g*16+nb] = bias_g[nb*128+p]
    # g order: r_sum, z_sum, ih2, hh2, neg_z_sum (unused).  hh2 is xSW
    # because it adds to the SW-scaled PSUM before the tanh descale.
    bias_all = np.stack(
        [bias_ih[0] + bias_hh[0], bias_ih[1] + bias_hh[1], bias_ih[2],
         np.float32(sw) * bias_hh[2], -(bias_ih[1] + bias_hh[1])]
    )  # [5, H]
    bpack = np.ascontiguousarray(
        bias_all.reshape(5, NNB, P).transpose(2, 0, 1).reshape(P, 5 * NNB)
    )

    def t_pack(a, dt):
        # [BL, H] -> [P, NKB, BL] with [kp, k, m] = a[m, k*128+kp]
        return np.ascontiguousarray(
            a.T.reshape(NKB, P, BL).transpose(1, 0, 2).astype(dt)
        )

    in_maps = []
    for c in range(N_CORES):
        sl = slice(c * BL, (c + 1) * BL)
        in_maps.append(
            {
                "xt8": t_pack(input[sl], ml_dtypes.float8_e4m3),
                "hxt8": t_pack(hx[sl], ml_dtypes.float8_e4m3),
                "xtb": np.ascontiguousarray(
                    t_pack(input[sl], ml_dtypes.bfloat16)[:, :NBF, :]
                ),
                "hxt32": t_pack(hx[sl], np.float32),
                "w8": w8p,
                "w16": w16p,
                "w8n": w8np,
                "w8nh": w8nhp,
                "w8nf": w8nfp,
                "b": bpack,
            }
        )
    return in_maps


_PROGRAM_CACHE = {}


def kernel(input, hx, weight_ih, weight_hh, bias_ih, bias_hh, _trace=False):
    wmax = float(
        max(
            np.abs(np.asarray(weight_ih, dtype=np.float32)).max(),
            np.abs(np.asarray(weight_hh, dtype=np.float32)).max(),
        )
    )
    sw = SWMAX / wmax if wmax > 0 else 64.0
    key = round(sw, 6)
    if key not in _PROGRAM_CACHE:
        _PROGRAM_CACHE[key] = _build_program(sw)
    nc = _PROGRAM_CACHE[key]
    in_maps = _pack_inputs(input, hx, weight_ih, weight_hh, bias_ih, bias_hh, sw)
    res = run_bass_kernel_spmd(nc, in_maps, list(range(N_CORES)), trace=_trace)
    out = np.empty((B, H), dtype=np.float32)
    for c in range(N_CORES):
        out[c * BL : (c + 1) * BL] = res.results[c]["out"].T
    if _trace:
        kernel.last_exec_time_ns = res.exec_time_ns
    return out
